# revision 1
# baseline (speedup 1.0000x reference)
"""Trainium2 Bass kernel for geodesic convolution (gnn_message_passing).

Computation (per vertex v):
  x[v,i,j,c]  = sum_t bary_w[v,i,j,t] * signal[bary_idx[v,i,j,t], c]
  conv[v,k,d] = sum_{i,j,c} x[v,i,j,c] * K[i,(j+k)%NT,c,d]
  out[v,:]    = relu(conv[v, argmax_k ||conv[v,k,:]||, :])

Strategy: shard V across 8 cores (data-parallel). Per core, per tile of 128
vertices: dma_gather of the 3*5*8 = 120 signal rows per vertex (v-major
layout; int16 indices biased by -32768 against a mid-tensor base so the
signed offsets cover all 50000 rows), DVE weighted sum over the 3 barycentric
taps, PE transpose of x to channel-major, one accumulated matmul chain
against the pre-rotated kernel matrix W[(i,j,c),(k,d)], then
norms/argmax/select/relu epilogue on DVE.
"""

import numpy as np

# Problem constants (hardcoded; kernel.py must be self-contained).
V, NR, NT, CIN, COUT = 50000, 5, 8, 64, 64
NCORES = 8
VPC = V // NCORES            # 6250 vertices per core
TPT = 128                    # vertices per tile (partition dim)
NTILES = -(-VPC // TPT)      # 49
VPAD = NTILES * TPT          # 6272
IJ = NR * NT                 # 40
E = IJ * 3                   # 120 gathered rows per vertex
EP = E + 1                   # +1 pad slot per partition (trailing-trim guard)
NIDX = EP * TPT              # 15488 gather indices per tile
NS = NIDX // 16              # idx free dim in wrapped-16 layout
KC = IJ * CIN                # 2560 contraction dim
NCHUNK = KC // 128           # 20
ND = NT * COUT               # 512 output cols (k,d)

_CACHE = {}


def build_program(ntiles=NTILES, v_src=V, repeat=1):
    """Build the Bacc program for one SPMD core. Returns compiled nc.

    repeat > 1 duplicates the whole tile loop (same inputs/outputs) for
    wall-clock slope timing; the extra passes just overwrite the outputs.
    """
    import concourse.bass as bass
    import concourse.mybir as mybir
    import concourse.tile as tile
    from concourse import bacc
    from concourse.masks import make_identity

    f32 = mybir.dt.float32
    i16 = mybir.dt.int16

    base = 32768 if v_src > 32768 else 0

    nc = bacc.Bacc(
        "TRN2",
        target_bir_lowering=False,
        debug=False,
        enable_asserts=False,
        num_devices=NCORES,
    )
    vpad = ntiles * TPT
    sig_d = nc.dram_tensor("signal", [v_src, CIN], f32, kind="ExternalInput")
    wv_d = nc.dram_tensor("wv", [vpad, E], f32, kind="ExternalInput")
    idx_d = nc.dram_tensor("idx16", [ntiles * 128, NS], i16, kind="ExternalInput")
    wm_d = nc.dram_tensor("wm", [KC, ND], f32, kind="ExternalInput")
    out_d = nc.dram_tensor("out", [vpad, COUT], f32, kind="ExternalOutput")

    sig_base = sig_d.ap()[base:, :] if base else sig_d.ap()

    with tile.TileContext(nc) as tc:
        with (
            tc.tile_pool(name="const", bufs=1) as cpool,
            tc.tile_pool(name="io", bufs=2) as iopool,
            tc.tile_pool(name="g", bufs=3) as gpool,
            tc.tile_pool(name="x", bufs=2) as xpool,
            tc.tile_pool(name="xT", bufs=3) as xtpool,
            tc.tile_pool(name="epi", bufs=2) as epool,
            tc.tile_pool(name="psA", bufs=2, space="PSUM") as psA,
            tc.tile_pool(name="psB", bufs=3, space="PSUM") as psB,
        ):
            # Resident: rotated kernel matrix [128, NCHUNK, 512] (chunk k of
            # contraction rows at [:, k, :]) and transpose identity.
            wm_t = cpool.tile([128, NCHUNK, ND], f32)
            nc.sync.dma_start(
                out=wm_t[:],
                in_=wm_d.ap().rearrange("(k p) n -> p k n", p=128),
            )
            ident = cpool.tile([128, 128], f32)
            make_identity(nc, ident[:])

            for it_rep in range(ntiles * repeat):
                it = it_rep % ntiles
                rows = slice(it * TPT, (it + 1) * TPT)
                w_t = iopool.tile([128, E], f32, tag="w")
                i_t = iopool.tile([128, NS], i16, tag="i")
                nc.sync.dma_start(out=w_t[:], in_=wv_d.ap()[rows, :])
                nc.sync.dma_start(out=i_t[:], in_=idx_d.ap()[rows, :])

                # Gather: g[p, e, :] = signal[idx[v_p, e], :]  (e < E; slot E is pad)
                g_t = gpool.tile([128, EP, CIN], f32)
                nc.gpsimd.dma_gather(
                    out_ap=g_t[:], in_ap=sig_base, idxs_ap=i_t[:],
                    num_idxs=NIDX, num_idxs_reg=NIDX, elem_size=CIN,
                    single_packet=False,
                )

                # Weighted sum over the 3 barycentric taps.
                g_e = g_t[:, :E, :]
                nc.vector.tensor_tensor(
                    out=g_e,
                    in0=g_e,
                    in1=w_t[:].unsqueeze(-1).to_broadcast([128, E, CIN]),
                    op=mybir.AluOpType.mult,
                )
                g4 = g_e.rearrange("p (ij t) c -> p ij t c", t=3)
                x_t = xpool.tile([128, IJ, CIN], f32)
                nc.vector.tensor_tensor(
                    out=x_t[:], in0=g4[:, :, 0, :], in1=g4[:, :, 1, :],
                    op=mybir.AluOpType.add,
                )
                nc.vector.tensor_tensor(
                    out=x_t[:], in0=x_t[:], in1=g4[:, :, 2, :],
                    op=mybir.AluOpType.add,
                )

                # Transpose x to channel-major and run the matmul chain.
                x2 = x_t[:].rearrange("p ij c -> p (ij c)")
                conv_p = psA.tile([128, ND], f32, tag="conv")
                for k in range(NCHUNK):
                    pt = psB.tile([128, 128], f32, tag="pt")
                    nc.tensor.transpose(
                        pt[:], x2[:, k * 128:(k + 1) * 128], ident[:]
                    )
                    xT = xtpool.tile([128, 128], f32, tag="xT")
                    nc.scalar.copy(out=xT[:], in_=pt[:])
                    nc.tensor.matmul(
                        conv_p[:],
                        lhsT=xT[:],
                        rhs=wm_t[:, k, :],
                        start=(k == 0),
                        stop=(k == NCHUNK - 1),
                    )

                # Epilogue: norms over d, argmax over k (via is_equal mask),
                # masked-sum select, relu.
                sq_t = epool.tile([128, ND], f32, tag="sq")
                nc.scalar.activation(
                    out=sq_t[:], in_=conv_p[:],
                    func=mybir.ActivationFunctionType.Square,
                )
                norm_t = epool.tile([128, NT], f32, tag="norm")
                nc.vector.tensor_reduce(
                    out=norm_t[:],
                    in_=sq_t[:].rearrange("p (k d) -> p k d", d=COUT),
                    axis=mybir.AxisListType.X,
                    op=mybir.AluOpType.add,
                )
                mx_t = epool.tile([128, 1], f32, tag="mx")
                nc.vector.tensor_reduce(
                    out=mx_t[:], in_=norm_t[:],
                    axis=mybir.AxisListType.X, op=mybir.AluOpType.max,
                )
                mask_t = epool.tile([128, NT], f32, tag="mask")
                nc.vector.tensor_scalar(
                    out=mask_t[:], in0=norm_t[:], scalar1=mx_t[:], scalar2=None,
                    op0=mybir.AluOpType.is_equal,
                )
                msel_t = epool.tile([128, NT, COUT], f32, tag="msel")
                nc.vector.tensor_tensor(
                    out=msel_t[:],
                    in0=conv_p[:].rearrange("p (k d) -> p k d", d=COUT),
                    in1=mask_t[:].unsqueeze(-1).to_broadcast([128, NT, COUT]),
                    op=mybir.AluOpType.mult,
                )
                o_t = epool.tile([128, COUT], f32, tag="o")
                nc.vector.tensor_reduce(
                    out=o_t[:],
                    in_=msel_t[:].rearrange("p k d -> p d k"),
                    axis=mybir.AxisListType.X,
                    op=mybir.AluOpType.add,
                )
                nc.vector.tensor_scalar_max(o_t[:], o_t[:], 0.0)
                nc.sync.dma_start(out=out_d.ap()[rows, :], in_=o_t[:])

    nc.compile()
    return nc


def make_idx16(idx_vp, ntiles, base):
    """[vpad, E] int32 row indices -> [ntiles*128, NS] wrapped int16.

    Gather list position n = e*128 + p must hold idx[tile*128 + p, e]; the
    ucode reads logical position i from wrapped[i % 16, i // 16], replicated
    across the 8 Q7 cores (16 partitions each). Slot E is a pad row of
    positive indices so the ucode's trailing-negative trim never fires.
    """
    vpad = ntiles * TPT
    out = np.empty((ntiles, 128, NS), np.int16)
    for t in range(ntiles):
        blk = idx_vp[t * TPT:(t + 1) * TPT]          # [128, E]
        lst = np.full(NIDX, 1, np.int32)             # pad slots -> row base+1
        lst[: E * 128] = (blk.T.astype(np.int32) - base).ravel()  # n = e*128+p
        w = lst.reshape(-1, 16).T                     # [16, NIDX/16]
        out[t] = np.tile(w, (8, 1))
    return out.reshape(ntiles * 128, NS)


def _host_prep(signal, bary_w, bary_idx, kernel):
    """Build per-core input maps. All host-side numpy, not timed."""
    jj = np.arange(NT)
    rot = kernel[:, (jj[:, None] + jj[None, :]) % NT, :, :]  # [i,j,k,c,d]
    wm = np.ascontiguousarray(
        rot.transpose(0, 1, 3, 2, 4).reshape(KC, ND), dtype=np.float32
    )
    sig = np.ascontiguousarray(signal, dtype=np.float32)
    base = 32768
    wv_full = bary_w.reshape(V, E).astype(np.float32)
    idx_full = bary_idx.reshape(V, E).astype(np.int32)
    in_maps = []
    for c in range(NCORES):
        sl = slice(c * VPC, (c + 1) * VPC)
        wv = np.zeros((VPAD, E), np.float32)
        wv[:VPC] = wv_full[sl]
        idx = np.zeros((VPAD, E), np.int32)
        idx[:VPC] = idx_full[sl]
        in_maps.append({
            "signal": sig,
            "wv": wv,
            "idx16": make_idx16(idx, NTILES, base),
            "wm": wm,
        })
    return in_maps


def kernel(signal, bary_w, bary_idx, kernel):
    from concourse.bass_utils import run_bass_kernel_spmd

    if "nc" not in _CACHE:
        _CACHE["nc"] = build_program()
    nc = _CACHE["nc"]
    in_maps = _host_prep(signal, bary_w, bary_idx, kernel)
    res = run_bass_kernel_spmd(nc, in_maps, core_ids=list(range(NCORES)))
    out = np.concatenate(
        [res.results[c]["out"][:VPC] for c in range(NCORES)], axis=0
    )
    return out.astype(np.float32)



# revision 3
# speedup vs baseline: 1.2818x; 1.2818x over previous
"""Trainium2 Bass kernel for geodesic convolution (gnn_message_passing).

Computation (per vertex v):
  x[v,ij,c]   = sum_t bary_w[v,ij,t] * signal[bary_idx[v,ij,t], c]
  conv[v,r,d] = sum_{ij,c} x[v,ij,c] * K[i(ij),(j(ij)+r)%NT,c,d]
  out[v,:]    = relu(conv[v, argmax_r ||conv[v,r,:]||, :])

Strategy: shard V across 8 cores. The signal lives resident in SBUF as an
fp16 pair table: table[p, e] = (sig[e, c(p)], sig[e+25000, c(p)]) packed in
one 32-bit word, which keeps ap_gather's num_elems under the 2^15 cap while
covering all 50000 rows. Slots are split across partition halves by ij
parity (partitions 0-63 even ij, 64-127 odd ij, channel = p%64) so the
matmul keeps a full K=128 contraction. Per 32-vertex quarter-tile, GPSIMD
ap_gather pulls one word per (ij,t,v) slot; host-prepared f32 weights carry
a zero in the wrong vertex-half lane, so the DVE multiply (f32, in place
over the weights) + tap/lane adds produce x in f32 channel-major. fp16 is
only used for the signal values themselves - everything downstream is f32,
which keeps the rotation-argmax flips rare enough for the 2e-2 gate.
20 accumulating f32 matmuls per 128-vertex tile read the rotated kernel
from a j-duplicated (rotation-deduped, parity-shifted) resident table, then
the norms/argmax/select/relu epilogue runs per tile.
"""

import numpy as np

# Problem constants (hardcoded; kernel.py must be self-contained).
V, NR, NT, CIN, COUT = 50000, 5, 8, 64, 64
NCORES = 8
VPC = V // NCORES            # 6250 vertices per core
TPT = 128                    # vertices per PE tile
NTILES = -(-VPC // TPT)      # 49
VPAD = NTILES * TPT          # 6272
HALFV = V // 2               # 25000 pair-table entries
IJ = NR * NT                 # 40
KP = IJ // 2                 # 20 ij-pairs (matmul chunks)
NQ = 4                       # gather quarters per PE tile
QT = TPT // NQ               # 32 vertices per quarter
NSLOTQ = KP * 3 * QT         # 1920 gather slots per partition-group
NWQ = NSLOTQ * 2             # 3840 weight lanes per group
NIWQ = NSLOTQ // 16          # 120 wrapped idx free dim
ND = NT * COUT               # 512 output cols (r,d)

_CACHE = {}


def build_program(ntiles=NTILES, repeat=1):
    """Build the Bacc program for one SPMD core. Returns compiled nc.

    repeat > 1 duplicates the whole tile loop (same inputs/outputs) for
    wall-clock slope timing; the extra passes just overwrite the outputs.
    """
    import concourse.bass as bass
    import concourse.mybir as mybir
    import concourse.tile as tile
    from concourse import bacc

    f32 = mybir.dt.float32
    f16 = mybir.dt.float16
    i16 = mybir.dt.int16

    nc = bacc.Bacc(
        "TRN2",
        target_bir_lowering=False,
        debug=False,
        enable_asserts=False,
        num_devices=NCORES,
    )
    sig_d = nc.dram_tensor("sigp", [128, HALFV, 2], f16, kind="ExternalInput")
    kd_d = nc.dram_tensor("kdup", [128, NR, 16 * COUT], f32, kind="ExternalInput")
    idx_d = nc.dram_tensor(
        "idx16", [ntiles * NQ * 128, NIWQ], i16, kind="ExternalInput"
    )
    wb_d = nc.dram_tensor("wb2", [ntiles, NQ, 2, NWQ], f32, kind="ExternalInput")
    out_d = nc.dram_tensor("out", [ntiles * TPT, COUT], f32, kind="ExternalOutput")

    with tile.TileContext(nc) as tc:
        with (
            tc.tile_pool(name="const", bufs=1) as cpool,
            tc.tile_pool(name="io", bufs=2) as iopool,
            tc.tile_pool(name="w", bufs=2) as wpool,
            tc.tile_pool(name="g", bufs=2) as gpool,
            tc.tile_pool(name="x", bufs=2) as xpool,
            tc.tile_pool(name="epi", bufs=2) as epool,
            tc.tile_pool(name="ps", bufs=2, space="PSUM") as psA,
        ):
            table = cpool.tile([128, HALFV, 2], f16)
            nc.sync.dma_start(out=table[:], in_=sig_d.ap())
            kd_t = cpool.tile([128, NR, 16 * COUT], f32)
            nc.sync.dma_start(out=kd_t[:], in_=kd_d.ap())

            for it_rep in range(ntiles * repeat):
                it = it_rep % ntiles
                x_t = xpool.tile([128, KP, TPT], f32, tag="x")
                for q in range(NQ):
                    row0 = (it * NQ + q) * 128
                    i_t = iopool.tile([128, NIWQ], i16, tag="i")
                    nc.sync.dma_start(
                        out=i_t[:], in_=idx_d.ap()[row0:row0 + 128, :]
                    )
                    w_t = wpool.tile([128, NSLOTQ, 2], f32, tag="w")
                    wflat = w_t[:].rearrange("p n l -> p (n l)")
                    for parity in range(2):
                        nc.sync.dma_start(
                            out=wflat[parity * 64:(parity + 1) * 64, :],
                            in_=wb_d.ap()[it, q, parity, :]
                            .unsqueeze(0)
                            .to_broadcast([64, NWQ]),
                        )
                    g_t = gpool.tile([128, NSLOTQ, 2], f16, tag="g")
                    nc.gpsimd.ap_gather(
                        out_ap=g_t[:],
                        in_ap=table[:],
                        idxs_ap=i_t[:],
                        channels=128,
                        num_elems=HALFV,
                        d=2,
                        num_idxs=NSLOTQ,
                    )
                    # xw = w * g in f32, in place over the weights.
                    gflat = g_t[:].rearrange("p n l -> p (n l)")
                    nc.vector.tensor_tensor(
                        out=wflat, in0=wflat, in1=gflat,
                        op=mybir.AluOpType.mult,
                    )
                    # Tap sum over t3 (in place into t=0 slot), then lane sum
                    # into the assembled x tile.
                    g5 = w_t[:].rearrange("p (k t v) l -> p k t v l", k=KP, t=3)
                    nc.vector.tensor_tensor(
                        out=g5[:, :, 0], in0=g5[:, :, 0], in1=g5[:, :, 1],
                        op=mybir.AluOpType.add,
                    )
                    nc.vector.tensor_tensor(
                        out=g5[:, :, 0], in0=g5[:, :, 0], in1=g5[:, :, 2],
                        op=mybir.AluOpType.add,
                    )
                    nc.vector.tensor_tensor(
                        out=x_t[:, :, q * QT:(q + 1) * QT],
                        in0=g5[:, :, 0, :, 0],
                        in1=g5[:, :, 0, :, 1],
                        op=mybir.AluOpType.add,
                    )

                conv_p = psA.tile([128, ND], f32, tag="conv")
                for k in range(KP):
                    j0 = (2 * k) % NT
                    i0 = (2 * k) // NT
                    nc.tensor.matmul(
                        conv_p[:],
                        lhsT=x_t[:, k, :],
                        rhs=kd_t[:, i0, j0 * COUT:j0 * COUT + ND],
                        start=(k == 0),
                        stop=(k == KP - 1),
                    )

                # Epilogue: norms over d, argmax over r (via is_equal mask),
                # masked-sum select, relu.
                sq_t = epool.tile([128, ND], f32, tag="sq")
                nc.scalar.activation(
                    out=sq_t[:], in_=conv_p[:],
                    func=mybir.ActivationFunctionType.Square,
                )
                norm_t = epool.tile([128, NT], f32, tag="norm")
                nc.vector.tensor_reduce(
                    out=norm_t[:],
                    in_=sq_t[:].rearrange("p (r d) -> p r d", d=COUT),
                    axis=mybir.AxisListType.X,
                    op=mybir.AluOpType.add,
                )
                mx_t = epool.tile([128, 1], f32, tag="mx")
                nc.vector.tensor_reduce(
                    out=mx_t[:], in_=norm_t[:],
                    axis=mybir.AxisListType.X, op=mybir.AluOpType.max,
                )
                mask_t = epool.tile([128, NT], f32, tag="mask")
                nc.vector.tensor_scalar(
                    out=mask_t[:], in0=norm_t[:], scalar1=mx_t[:], scalar2=None,
                    op0=mybir.AluOpType.is_equal,
                )
                msel_t = epool.tile([128, NT, COUT], f32, tag="sq")
                nc.vector.tensor_tensor(
                    out=msel_t[:],
                    in0=conv_p[:].rearrange("p (r d) -> p r d", d=COUT),
                    in1=mask_t[:].unsqueeze(-1).to_broadcast([128, NT, COUT]),
                    op=mybir.AluOpType.mult,
                )
                o_t = epool.tile([128, COUT], f32, tag="o")
                nc.vector.tensor_reduce(
                    out=o_t[:],
                    in_=msel_t[:].rearrange("p r d -> p d r"),
                    axis=mybir.AxisListType.X,
                    op=mybir.AluOpType.add,
                )
                nc.vector.tensor_scalar_max(o_t[:], o_t[:], 0.0)
                nc.sync.dma_start(
                    out=out_d.ap()[it * TPT:(it + 1) * TPT, :], in_=o_t[:]
                )

    nc.compile()
    return nc


def _host_prep(signal, bary_w, bary_idx, kernel, ntiles=NTILES):
    """Build per-core input maps. All host-side numpy, not timed."""
    kern = np.asarray(kernel, np.float32)

    # j-duplicated, parity-shifted rotated kernel:
    # kdup[p, i, jj*64+d] = K[i, (jj + p//64) % NT, p%64, d], jj in [0,16).
    jj16 = np.arange(16)
    parts = []
    for par in range(2):
        kdp = kern[:, (jj16 + par) % NT, :, :]        # [i, jj, c, d]
        parts.append(kdp.transpose(2, 0, 1, 3))        # [c, i, jj, d]
    kdup = np.ascontiguousarray(
        np.concatenate(parts, axis=0).reshape(128, NR, 16 * COUT)
    )

    # Signal pair table: table[p, e] = (sig[e, c], sig[e+HALFV, c]), c = p%64.
    sb = np.asarray(signal).astype(np.float16)         # [V, 64]
    sigp = np.empty((128, HALFV, 2), np.float16)
    sigp[:, :, 0] = np.tile(sb[:HALFV].T, (2, 1))
    sigp[:, :, 1] = np.tile(sb[HALFV:].T, (2, 1))

    vpad = ntiles * TPT
    idxf = np.asarray(bary_idx).reshape(V, IJ, 3)
    wff = np.asarray(bary_w).reshape(V, IJ, 3).astype(np.float32)
    in_maps = []
    for c in range(NCORES):
        sl = slice(c * VPC, (c + 1) * VPC)
        n = min(VPC, vpad)
        idx = np.zeros((vpad, IJ, 3), np.int32)
        idx[:n] = idxf[sl][:n]
        w = np.zeros((vpad, IJ, 3), np.float32)
        w[:n] = wff[sl][:n]
        # [t, q, v32, k, parity, t3] -> [t, q, parity, k, t3, v32]
        idx_r = idx.reshape(ntiles, NQ, QT, KP, 2, 3).transpose(0, 1, 4, 3, 5, 2)
        w_r = w.reshape(ntiles, NQ, QT, KP, 2, 3).transpose(0, 1, 4, 3, 5, 2)
        e = (idx_r % HALFV).astype(np.int16)
        lane = idx_r // HALFV
        wb2 = np.stack(
            [w_r * (lane == 0), w_r * (lane == 1)], axis=-1
        )                                              # [t, q, 2, k, 3, v32, 2]
        wb2 = np.ascontiguousarray(wb2.reshape(ntiles, NQ, 2, NWQ), np.float32)
        # Wrapped idx: list position i at [i%16, i//16]; 4 copies per half.
        el = e.reshape(ntiles, NQ, 2, NIWQ, 16)
        wr = np.swapaxes(el, -1, -2)                   # [t, q, parity, 16, NIWQ]
        idx16 = np.empty((ntiles, NQ, 128, NIWQ), np.int16)
        idx16[:, :, 0:64] = np.tile(wr[:, :, 0], (1, 1, 4, 1))
        idx16[:, :, 64:128] = np.tile(wr[:, :, 1], (1, 1, 4, 1))
        in_maps.append({
            "sigp": sigp,
            "kdup": kdup,
            "idx16": np.ascontiguousarray(
                idx16.reshape(ntiles * NQ * 128, NIWQ)
            ),
            "wb2": wb2,
        })
    return in_maps


def kernel(signal, bary_w, bary_idx, kernel):
    from concourse.bass_utils import run_bass_kernel_spmd

    if "nc" not in _CACHE:
        _CACHE["nc"] = build_program()
    nc = _CACHE["nc"]
    in_maps = _host_prep(signal, bary_w, bary_idx, kernel)
    res = run_bass_kernel_spmd(nc, in_maps, core_ids=list(range(NCORES)))
    out = np.concatenate(
        [res.results[c]["out"][:VPC] for c in range(NCORES)], axis=0
    )
    return out.astype(np.float32)
